# revision 18
# baseline (speedup 1.0000x reference)
"""Grouped-GEMM MoE kernel for Trainium2, expert-parallel across 8 NeuronCores.

Strategy (matches the module's expert-parallel path):
  - Host routes: sort the T*topk routed pairs by expert id; core e gets the
    tokens routed to expert e (padded to a common capacity C).
  - Device (per core): Y = gelu(X @ W1 + b1) @ W2 + b2, scaled per-row by the
    routing weight. Two chained GEMMs on the PE array in bf16 with fp32 PSUM
    accumulation; gelu fused into the PSUM->SBUF eviction on the ACT engine.
  - Host combines: scatter rows back by pair index and sum the topk=2 slots.

Perf notes (trace-driven):
  - Front DMAs are issued in need-time order across two HWDGE paths (Sync +
    Scalar) with the critical first chunks small, so the first matmul starts
    ~11us into the kernel (vs ~34us with one monolithic weight DMA) and the
    PE then runs gap-free at the bf16 streaming roofline (~213ns per
    128x128x512 matmul).
  - Capacity is padded to a multiple of 128 (not 512); the remainder tokens
    ride along the last full tile's GEMM1 k-chains into a second PSUM bank.
  - b2 is applied on the host (exact, and it is zero in this module anyway);
    outputs are stored as bf16, halving the output DMA traffic.

Problem shapes (hardcoded per contract): B=4, S=4096, H=1024, F=2048, E=8,
TOPK=2.
"""

import sys

for _p in ("/opt/trn_rl_repo", "/opt/pypackages"):
    if _p not in sys.path:
        sys.path.insert(0, _p)

import ml_dtypes
import numpy as np

import concourse.bass as bass  # noqa: F401  (engine types come via bacc)
import concourse.mybir as mybir
import concourse.tile as tile
from concourse import bacc
from concourse.bass_utils import run_bass_kernel_spmd

H = 1024
F = 2048
E = 8
TOPK = 2
N_CORES = 8
P = 128
NTILE = 512          # full token-tile width (matmul moving dim)
KK1 = H // P         # 8  k-steps in GEMM1
KK2 = F // P         # 16 k-steps in GEMM2
MT1 = F // P         # 16 output m-tiles in GEMM1
NT2 = H // NTILE     # 2  output n-tiles in GEMM2

BF16 = mybir.dt.bfloat16
F32 = mybir.dt.float32

_CACHE = {}
last_result = None   # BassKernelResults of the most recent device run


def _build(C):
    """Build + compile the per-core program for capacity C (multiple of 16)."""
    assert C % 16 == 0
    nfull = C // NTILE            # full 512-token tiles
    rem = C - nfull * NTILE       # remainder width (< 512, any multiple of 16)
    # The remainder rides along with the last full tile: each GEMM1 k-step
    # loads the stationary weight once and streams 512+rem tokens through it
    # as two chained matmuls into separate PSUM banks (the second LDWEIGHTS
    # is pulled into the background weight buffer under the 512-wide stream,
    # so the rider tokens cost only their streaming cycles).
    if nfull == 0:
        widths = [rem]
    elif rem:
        widths = [NTILE] * (nfull - 1) + [NTILE + rem]
    else:
        widths = [NTILE] * nfull
    nmc = -(-C // P)              # token m-chunks (GEMM2 output rows)

    nc = bacc.Bacc("TRN2", target_bir_lowering=False, debug=False,
                   num_devices=N_CORES)

    # DRAM I/O. Layouts are chosen so every DMA is a plain strided copy:
    #   xt[p, kk, c]       = X[c, kk*128+p]   (tokens transposed, H in 8x128)
    #   w1[p, m, kk*128+j] = W1[kk*128+p, m*128+j]  (per-m contiguous chunks)
    #   w2[p, kk, j]       = W2[kk*128+p, j]
    #   b1[p, m]           = b1[m*128+p]
    #   rw[p, mc]          = rweight[mc*128+p]
    #   y[mc, p, j]        = Y[mc*128+p, j]   (bf16)
    xt_d = nc.dram_tensor("xt", [P, KK1, C], BF16, kind="ExternalInput")
    w1_d = nc.dram_tensor("w1", [P, MT1, KK1 * P], BF16, kind="ExternalInput")
    w2_d = nc.dram_tensor("w2", [P, KK2, H], BF16, kind="ExternalInput")
    b1_d = nc.dram_tensor("b1", [P, MT1], F32, kind="ExternalInput")
    rw_d = nc.dram_tensor("rw", [P, nmc], F32, kind="ExternalInput")
    y_d = nc.dram_tensor("y", [nmc, P, H], BF16, kind="ExternalOutput")

    with tile.TileContext(nc) as tc:
        with (
            tc.tile_pool(name="const", bufs=1) as const,
            tc.tile_pool(name="xin", bufs=3) as xin,
            tc.tile_pool(name="gact", bufs=3) as gact,
            tc.tile_pool(name="yout", bufs=4) as yout,
            tc.tile_pool(name="psg", bufs=3, space="PSUM") as psg,
            tc.tile_pool(name="psgr", bufs=2, space="PSUM") as psgr,
            tc.tile_pool(name="psy", bufs=3, space="PSUM") as psy,
        ):
            # First x tile in 4 chunks (2 k-slices each) so GEMM1 m=0 can
            # start as soon as the first 256 KB lands. DMA issue on the Sync
            # engine serializes at ~0.6us per dma_start, so the front only
            # carries the chunks that actually gate compute; the bulk of the
            # weights goes out as a few large transfers.
            # Front DMA schedule. Issue order doubles as packet priority in
            # the SDMA engine FIFOs, so transfers are issued strictly in
            # need-time order and the bulk weights go out last. Sync engine:
            # the xt0 chunks, then the tail of W1, then rw, then W2 (split so
            # its packets queue behind the critical front and the 9th+ sync
            # DMA is gated by queue reuse). Scalar engine (parallel HWDGE
            # path): w1 m0-3 + b1, which gate the first matmuls/eviction.
            # PE clock prewarm: the HAM clock gate needs ~3.4us of sustained
            # matmul activity to lift the PE from 1.2 to 2.4 GHz. Run dummy
            # matmuls on a memset tile while the first input DMAs are in
            # flight so the real matmuls start at full clock.
            dum = const.tile([P, NTILE], BF16)
            nc.gpsimd.memset(dum[:], 0.0)
            pyd = psy.tile([P, NTILE], F32, tag="py")
            for _ in range(12):
                nc.tensor.matmul(pyd[:], dum[:, :P], dum[:],
                                 start=True, stop=True)

            xt0 = xin.tile([P, KK1, NTILE if widths else P], BF16, tag="xt")
            for q in range(4):
                nc.sync.dma_start(xt0[:, 2 * q:2 * q + 2, :],
                                  xt_d[:, 2 * q:2 * q + 2, :widths[0]])

            w1_t = []
            for m in range(4):
                t = const.tile([P, KK1 * P], BF16, tag=f"w1_{m}")
                nc.scalar.dma_start(t[:], w1_d[:, m, :])
                w1_t.append(t)
            b1_sb = const.tile([P, MT1], F32)
            nc.scalar.dma_start(b1_sb[:], b1_d[:])

            # Rest of W1 in two 1.5 MB transfers (needed ~22us+ in).
            w1g = []
            for g in range(2):
                t = const.tile([P, 6, KK1 * P], BF16, tag=f"w1g_{g}")
                nc.sync.dma_start(t[:], w1_d[:, 4 + 6 * g:10 + 6 * g, :])
                w1g.append(t)

            def w1s(m, kk):
                if m < 4:
                    return w1_t[m][:, kk * P:(kk + 1) * P]
                g, mi = divmod(m - 4, 6)
                return w1g[g][:, mi, kk * P:(kk + 1) * P]

            rw_sb = const.tile([P, nmc], F32)
            nc.sync.dma_start(rw_sb[:], rw_d[:])

            # W2 in four 1 MB chunks (needed only when GEMM2 starts ~45us).
            w2_sb = const.tile([P, KK2, H], BF16)
            for g in range(4):
                nc.sync.dma_start(w2_sb[:, 4 * g:4 * g + 4, :],
                                  w2_d[:, 4 * g:4 * g + 4, :])

            cbase = 0
            for ct, w in enumerate(widths):
                if ct == 0:
                    xt_sb = xt0
                else:
                    xt_sb = xin.tile([P, KK1, w], BF16, tag="xt")
                    nc.sync.dma_start(
                        xt_sb[:], xt_d[:, :, cbase:cbase + w])

                # GEMM1: GT[f, c] = sum_h W1[h, f] * XT[h, c], then
                # gelu(.+b1) on eviction. F on partitions, tokens on free.
                wf = min(w, NTILE)       # main chain width
                wr = w - wf              # rider chain width (last tile only)
                gt_sb = gact.tile([P, MT1, w], BF16, tag="gt")
                for m in range(MT1):
                    pg = psg.tile([P, wf], F32, tag="pg")
                    if wr:
                        pgr = psgr.tile([P, wr], F32, tag="pgr")
                    for kk in range(KK1):
                        nc.tensor.matmul(
                            pg[:],
                            w1s(m, kk),
                            xt_sb[:, kk, :wf],
                            start=(kk == 0), stop=(kk == KK1 - 1))
                        if wr:
                            nc.tensor.matmul(
                                pgr[:],
                                w1s(m, kk),
                                xt_sb[:, kk, wf:w],
                                start=(kk == 0), stop=(kk == KK1 - 1))
                    nc.scalar.activation(
                        gt_sb[:, m, :wf], pg[:],
                        mybir.ActivationFunctionType.Gelu,
                        bias=b1_sb[:, m:m + 1])
                    if wr:
                        nc.scalar.activation(
                            gt_sb[:, m, wf:w], pgr[:],
                            mybir.ActivationFunctionType.Gelu,
                            bias=b1_sb[:, m:m + 1])

                # GEMM2: Y[c, j] = sum_f GT[f, c] * W2[f, j]; tokens on
                # partitions. Evict: * routing_weight -> bf16, store.
                for mo in range(-(-w // P)):
                    pw = min(P, w - mo * P)   # last chunk may be partial
                    mc = cbase // P + mo
                    for n in range(NT2):
                        py = psy.tile([P, NTILE], F32, tag="py")
                        for kk in range(KK2):
                            nc.tensor.matmul(
                                py[:pw, :],
                                gt_sb[:, kk, mo * P:mo * P + pw],
                                w2_sb[:, kk, n * NTILE:(n + 1) * NTILE],
                                start=(kk == 0), stop=(kk == KK2 - 1))
                        yo = yout.tile([P, NTILE], BF16, tag="yo")
                        nc.vector.tensor_scalar_mul(
                            yo[:pw, :], py[:pw, :], rw_sb[:pw, mc:mc + 1])
                        nc.sync.dma_start(
                            y_d[mc, :pw, n * NTILE:(n + 1) * NTILE],
                            yo[:pw, :])
                cbase += w

    nc.compile()
    return nc


def kernel(hidden_states, expert_weights, top_experts, w1, b1, w2, b2,
           _trace=False):
    global last_result
    x = np.asarray(hidden_states, dtype=np.float32)
    fw = np.asarray(expert_weights, dtype=np.float32).reshape(-1)
    fe = np.asarray(top_experts).reshape(-1).astype(np.int64)
    w1 = np.asarray(w1, dtype=np.float32)
    b1 = np.asarray(b1, dtype=np.float32)
    w2 = np.asarray(w2, dtype=np.float32)
    b2 = np.asarray(b2, dtype=np.float32)

    b, s, h = x.shape
    T = b * s
    xf = x.reshape(T, h)
    npair = T * TOPK

    # Host-side routing: stable sort of pair indices by expert id.
    order = np.argsort(fe, kind="stable")
    counts = np.bincount(fe, minlength=E)
    starts = np.concatenate([[0], np.cumsum(counts)])
    C = max(int(-(-counts.max() // 16)) * 16, 16)
    nmc = -(-C // P)

    key = C
    if key not in _CACHE:
        _CACHE[key] = _build(C)
    nc = _CACHE[key]

    in_maps = []
    for e in range(E):
        idx = order[starts[e]:starts[e + 1]]
        cnt = len(idx)
        xe = np.zeros((C, H), np.float32)
        xe[:cnt] = xf[idx // TOPK]
        xt = np.ascontiguousarray(
            xe.T.reshape(KK1, P, C).transpose(1, 0, 2)).astype(
                ml_dtypes.bfloat16)
        rwe = np.zeros(nmc * P, np.float32)
        rwe[:cnt] = fw[idx]
        in_maps.append({
            "xt": xt,
            "w1": np.ascontiguousarray(
                w1[e].reshape(KK1, P, MT1, P).transpose(1, 2, 0, 3).reshape(
                    P, MT1, KK1 * P)).astype(ml_dtypes.bfloat16),
            "w2": np.ascontiguousarray(
                w2[e].reshape(KK2, P, H).transpose(1, 0, 2)).astype(
                    ml_dtypes.bfloat16),
            "b1": np.ascontiguousarray(b1[e].reshape(MT1, P).T),
            "rw": np.ascontiguousarray(rwe.reshape(nmc, P).T),
        })

    res = run_bass_kernel_spmd(nc, in_maps, list(range(N_CORES)),
                               trace=_trace)
    last_result = res

    routed = np.zeros((npair, H), np.float32)
    for e in range(E):
        idx = order[starts[e]:starts[e + 1]]
        cnt = len(idx)
        ye = np.asarray(res.results[e]["y"]).astype(
            np.float32).reshape(nmc * P, H)
        routed[idx] = ye[:cnt]
        if b2[e].any():
            # b2 is applied on the host (exact fp32): y += rw * b2[expert]
            routed[idx] += fw[idx][:, None] * b2[e][None, :]

    y = routed.reshape(T, TOPK, H).sum(axis=1)
    return y.reshape(b, s, h).astype(np.float32)


# revision 21
# speedup vs baseline: 1.1833x; 1.1833x over previous
"""Grouped-GEMM MoE kernel for Trainium2, expert-parallel across 8 NeuronCores.

Strategy (matches the module's expert-parallel path):
  - Host routes: sort the T*topk routed pairs by expert id; core e gets the
    tokens routed to expert e (padded to a common capacity C).
  - Device (per core): Y = gelu(X @ W1 + b1) @ W2 + b2, scaled per-row by the
    routing weight. Two chained GEMMs on the PE array in bf16 with fp32 PSUM
    accumulation; gelu fused into the PSUM->SBUF eviction on the ACT engine.
  - Host combines: scatter rows back by pair index and sum the topk=2 slots.

Perf notes (trace-driven):
  - Front DMAs are issued in need-time order across two HWDGE paths (Sync +
    Scalar) with the critical first chunks small, so the first matmul starts
    ~11us into the kernel (vs ~34us with one monolithic weight DMA) and the
    PE then runs gap-free at the bf16 streaming roofline (~213ns per
    128x128x512 matmul).
  - Capacity is padded to a multiple of 128 (not 512); the remainder tokens
    ride along the last full tile's GEMM1 k-chains into a second PSUM bank.
  - b2 is applied on the host (exact, and it is zero in this module anyway);
    outputs are stored as bf16, halving the output DMA traffic.

Problem shapes (hardcoded per contract): B=4, S=4096, H=1024, F=2048, E=8,
TOPK=2.
"""

import sys

for _p in ("/opt/trn_rl_repo", "/opt/pypackages"):
    if _p not in sys.path:
        sys.path.insert(0, _p)

import ml_dtypes
import numpy as np

import concourse.bass as bass  # noqa: F401  (engine types come via bacc)
import concourse.mybir as mybir
import concourse.tile as tile
from concourse import bacc
from concourse.bass_utils import run_bass_kernel_spmd

H = 1024
F = 2048
E = 8
TOPK = 2
N_CORES = 8
P = 128
NTILE = 512          # full token-tile width (matmul moving dim)
KK1 = H // P         # 8  k-steps in GEMM1
KK2 = F // P         # 16 k-steps in GEMM2
MT1 = F // P         # 16 output m-tiles in GEMM1
NT2 = H // NTILE     # 2  output n-tiles in GEMM2

BF16 = mybir.dt.bfloat16
F32 = mybir.dt.float32

_CACHE = {}
last_result = None   # BassKernelResults of the most recent device run


def _build(C):
    """Build + compile the per-core program for capacity C (multiple of 16)."""
    assert C % 16 == 0
    nfull = C // NTILE            # full 512-token tiles
    rem = C - nfull * NTILE       # remainder width (< 512, any multiple of 16)
    # The remainder rides along with the last full tile: each GEMM1 k-step
    # loads the stationary weight once and streams 512+rem tokens through it
    # as two chained matmuls into separate PSUM banks (the second LDWEIGHTS
    # is pulled into the background weight buffer under the 512-wide stream,
    # so the rider tokens cost only their streaming cycles).
    if nfull == 0:
        widths = [rem]
    elif rem:
        widths = [NTILE] * (nfull - 1) + [NTILE + rem]
    else:
        widths = [NTILE] * nfull
    nmc = -(-C // P)              # token m-chunks (GEMM2 output rows)

    nc = bacc.Bacc("TRN2", target_bir_lowering=False, debug=False,
                   num_devices=N_CORES)

    # DRAM I/O. Layouts are chosen so every DMA is a plain strided copy:
    #   xt[p, kk, c]       = X[c, kk*128+p]   (tokens transposed, H in 8x128)
    #   w1[p, m, kk*128+j] = W1[kk*128+p, m*128+j]  (per-m contiguous chunks)
    #   w2[p, kk, j]       = W2[kk*128+p, j]
    #   b1[p, m]           = b1[m*128+p]
    #   rw[p, mc]          = rweight[mc*128+p]
    #   y[mc, p, j]        = Y[mc*128+p, j]   (bf16)
    xt_d = nc.dram_tensor("xt", [P, KK1, C], BF16, kind="ExternalInput")
    w1_d = nc.dram_tensor("w1", [P, MT1, KK1 * P], BF16, kind="ExternalInput")
    w2_d = nc.dram_tensor("w2", [P, KK2, H], BF16, kind="ExternalInput")
    b1_d = nc.dram_tensor("b1", [P, MT1], F32, kind="ExternalInput")
    rw_d = nc.dram_tensor("rw", [P, nmc], F32, kind="ExternalInput")
    y_d = nc.dram_tensor("y", [nmc, P, H], BF16, kind="ExternalOutput")

    with tile.TileContext(nc) as tc:
        with (
            tc.tile_pool(name="const", bufs=1) as const,
            tc.tile_pool(name="xin", bufs=3) as xin,
            tc.tile_pool(name="gact", bufs=3) as gact,
            tc.tile_pool(name="yout", bufs=4) as yout,
            tc.tile_pool(name="psg", bufs=3, space="PSUM") as psg,
            tc.tile_pool(name="psgr", bufs=2, space="PSUM") as psgr,
            tc.tile_pool(name="psy", bufs=3, space="PSUM") as psy,
        ):
            # First x tile in 4 chunks (2 k-slices each) so GEMM1 m=0 can
            # start as soon as the first 256 KB lands. DMA issue on the Sync
            # engine serializes at ~0.6us per dma_start, so the front only
            # carries the chunks that actually gate compute; the bulk of the
            # weights goes out as a few large transfers.
            # Front DMA schedule. Issue order doubles as packet priority in
            # the SDMA engine FIFOs, so transfers are issued strictly in
            # need-time order and the bulk weights go out last. Sync engine:
            # the xt0 chunks, then the tail of W1, then rw, then W2 (split so
            # its packets queue behind the critical front and the 9th+ sync
            # DMA is gated by queue reuse). Scalar engine (parallel HWDGE
            # path): w1 m0-3 + b1, which gate the first matmuls/eviction.
            # PE clock prewarm: the HAM clock gate needs ~3.4us of sustained
            # matmul activity to lift the PE from 1.2 to 2.4 GHz. Run dummy
            # matmuls on a memset tile while the first input DMAs are in
            # flight so the real matmuls start at full clock.
            dum = const.tile([P, NTILE], BF16)
            nc.gpsimd.memset(dum[:], 0.0)
            pyd = psy.tile([P, NTILE], F32, tag="py")
            for _ in range(14):
                nc.tensor.matmul(pyd[:], dum[:, :P], dum[:],
                                 start=True, stop=True)

            xt0 = xin.tile([P, KK1, NTILE if widths else P], BF16, tag="xt")
            for q in range(4):
                nc.sync.dma_start(xt0[:, 2 * q:2 * q + 2, :],
                                  xt_d[:, 2 * q:2 * q + 2, :widths[0]])

            w1_t = []
            for m in range(4):
                t = const.tile([P, KK1 * P], BF16, tag=f"w1_{m}")
                nc.scalar.dma_start(t[:], w1_d[:, m, :])
                w1_t.append(t)
            b1_sb = const.tile([P, MT1], F32)
            nc.scalar.dma_start(b1_sb[:], b1_d[:])

            # Rest of W1 in two 1.5 MB transfers (needed ~22us+ in).
            w1g = []
            for g in range(2):
                t = const.tile([P, 6, KK1 * P], BF16, tag=f"w1g_{g}")
                nc.sync.dma_start(t[:], w1_d[:, 4 + 6 * g:10 + 6 * g, :])
                w1g.append(t)

            def w1s(m, kk):
                if m < 4:
                    return w1_t[m][:, kk * P:(kk + 1) * P]
                g, mi = divmod(m - 4, 6)
                return w1g[g][:, mi, kk * P:(kk + 1) * P]

            rw_sb = const.tile([P, nmc], F32)
            nc.sync.dma_start(rw_sb[:], rw_d[:])

            # W2 in four 1 MB chunks (needed only when GEMM2 starts ~45us).
            w2_sb = const.tile([P, KK2, H], BF16)
            for g in range(4):
                nc.sync.dma_start(w2_sb[:, 4 * g:4 * g + 4, :],
                                  w2_d[:, 4 * g:4 * g + 4, :])

            cbase = 0
            for ct, w in enumerate(widths):
                if ct == 0:
                    xt_sb = xt0
                else:
                    xt_sb = xin.tile([P, KK1, w], BF16, tag="xt")
                    nc.sync.dma_start(
                        xt_sb[:], xt_d[:, :, cbase:cbase + w])

                # GEMM1: GT[f, c] = sum_h W1[h, f] * XT[h, c], then
                # gelu(.+b1) on eviction. F on partitions, tokens on free.
                wf = min(w, NTILE)       # main chain width
                wr = w - wf              # rider chain width (last tile only)
                gt_sb = gact.tile([P, MT1, w], BF16, tag="gt")
                m_start = 0
                if ct == 0 and wr == 0:
                    # The front is DMA-bandwidth-bound: xt0 k-slices arrive
                    # about every 750ns while a single chain consumes one
                    # every 213ns. Interleave the first 4 m-chains across 4
                    # PSUM banks so each arriving slice is consumed 4x and
                    # the PE never outruns the chunked input DMAs.
                    NI = 4
                    pgs = [psg.tile([P, wf], F32, tag="pg", name=f"pgi{i}")
                           for i in range(NI - 1)]
                    pgs.append(psgr.tile([P, wf], F32, tag="pgr",
                                         name="pgi3"))
                    for kk in range(KK1):
                        for mi in range(NI):
                            nc.tensor.matmul(
                                pgs[mi][:],
                                w1s(mi, kk),
                                xt_sb[:, kk, :wf],
                                start=(kk == 0), stop=(kk == KK1 - 1))
                    for mi in range(NI):
                        nc.scalar.activation(
                            gt_sb[:, mi, :wf], pgs[mi][:],
                            mybir.ActivationFunctionType.Gelu,
                            bias=b1_sb[:, mi:mi + 1])
                    m_start = NI
                for m in range(m_start, MT1):
                    pg = psg.tile([P, wf], F32, tag="pg")
                    if wr:
                        pgr = psgr.tile([P, wr], F32, tag="pgr")
                    for kk in range(KK1):
                        nc.tensor.matmul(
                            pg[:],
                            w1s(m, kk),
                            xt_sb[:, kk, :wf],
                            start=(kk == 0), stop=(kk == KK1 - 1))
                        if wr:
                            nc.tensor.matmul(
                                pgr[:],
                                w1s(m, kk),
                                xt_sb[:, kk, wf:w],
                                start=(kk == 0), stop=(kk == KK1 - 1))
                    nc.scalar.activation(
                        gt_sb[:, m, :wf], pg[:],
                        mybir.ActivationFunctionType.Gelu,
                        bias=b1_sb[:, m:m + 1])
                    if wr:
                        nc.scalar.activation(
                            gt_sb[:, m, wf:w], pgr[:],
                            mybir.ActivationFunctionType.Gelu,
                            bias=b1_sb[:, m:m + 1])

                # GEMM2: Y[c, j] = sum_f GT[f, c] * W2[f, j]; tokens on
                # partitions. Evict: * routing_weight -> bf16, store.
                for mo in range(-(-w // P)):
                    pw = min(P, w - mo * P)   # last chunk may be partial
                    mc = cbase // P + mo
                    for n in range(NT2):
                        py = psy.tile([P, NTILE], F32, tag="py")
                        for kk in range(KK2):
                            nc.tensor.matmul(
                                py[:pw, :],
                                gt_sb[:, kk, mo * P:mo * P + pw],
                                w2_sb[:, kk, n * NTILE:(n + 1) * NTILE],
                                start=(kk == 0), stop=(kk == KK2 - 1))
                        yo = yout.tile([P, NTILE], BF16, tag="yo")
                        nc.vector.tensor_scalar_mul(
                            yo[:pw, :], py[:pw, :], rw_sb[:pw, mc:mc + 1])
                        nc.sync.dma_start(
                            y_d[mc, :pw, n * NTILE:(n + 1) * NTILE],
                            yo[:pw, :])
                cbase += w

    nc.compile()
    return nc


def kernel(hidden_states, expert_weights, top_experts, w1, b1, w2, b2,
           _trace=False):
    global last_result
    x = np.asarray(hidden_states, dtype=np.float32)
    fw = np.asarray(expert_weights, dtype=np.float32).reshape(-1)
    fe = np.asarray(top_experts).reshape(-1).astype(np.int64)
    w1 = np.asarray(w1, dtype=np.float32)
    b1 = np.asarray(b1, dtype=np.float32)
    w2 = np.asarray(w2, dtype=np.float32)
    b2 = np.asarray(b2, dtype=np.float32)

    b, s, h = x.shape
    T = b * s
    xf = x.reshape(T, h)
    npair = T * TOPK

    # Host-side routing: stable sort of pair indices by expert id.
    order = np.argsort(fe, kind="stable")
    counts = np.bincount(fe, minlength=E)
    starts = np.concatenate([[0], np.cumsum(counts)])
    C = max(int(-(-counts.max() // 16)) * 16, 16)
    nmc = -(-C // P)

    key = C
    if key not in _CACHE:
        _CACHE[key] = _build(C)
    nc = _CACHE[key]

    in_maps = []
    for e in range(E):
        idx = order[starts[e]:starts[e + 1]]
        cnt = len(idx)
        xe = np.zeros((C, H), np.float32)
        xe[:cnt] = xf[idx // TOPK]
        xt = np.ascontiguousarray(
            xe.T.reshape(KK1, P, C).transpose(1, 0, 2)).astype(
                ml_dtypes.bfloat16)
        rwe = np.zeros(nmc * P, np.float32)
        rwe[:cnt] = fw[idx]
        in_maps.append({
            "xt": xt,
            "w1": np.ascontiguousarray(
                w1[e].reshape(KK1, P, MT1, P).transpose(1, 2, 0, 3).reshape(
                    P, MT1, KK1 * P)).astype(ml_dtypes.bfloat16),
            "w2": np.ascontiguousarray(
                w2[e].reshape(KK2, P, H).transpose(1, 0, 2)).astype(
                    ml_dtypes.bfloat16),
            "b1": np.ascontiguousarray(b1[e].reshape(MT1, P).T),
            "rw": np.ascontiguousarray(rwe.reshape(nmc, P).T),
        })

    res = run_bass_kernel_spmd(nc, in_maps, list(range(N_CORES)),
                               trace=_trace)
    last_result = res

    routed = np.zeros((npair, H), np.float32)
    for e in range(E):
        idx = order[starts[e]:starts[e + 1]]
        cnt = len(idx)
        ye = np.asarray(res.results[e]["y"]).astype(
            np.float32).reshape(nmc * P, H)
        routed[idx] = ye[:cnt]
        if b2[e].any():
            # b2 is applied on the host (exact fp32): y += rw * b2[expert]
            routed[idx] += fw[idx][:, None] * b2[e][None, :]

    y = routed.reshape(T, TOPK, H).sum(axis=1)
    return y.reshape(b, s, h).astype(np.float32)


# revision 22
# speedup vs baseline: 1.1898x; 1.0055x over previous
"""Grouped-GEMM MoE kernel for Trainium2, expert-parallel across 8 NeuronCores.

Strategy (matches the module's expert-parallel path):
  - Host routes: sort the T*topk routed pairs by expert id; core e gets the
    tokens routed to expert e (padded to a common capacity C).
  - Device (per core): Y = gelu(X @ W1 + b1) @ W2 + b2, scaled per-row by the
    routing weight. Two chained GEMMs on the PE array in bf16 with fp32 PSUM
    accumulation; gelu fused into the PSUM->SBUF eviction on the ACT engine.
  - Host combines: scatter rows back by pair index and sum the topk=2 slots.

Perf notes (trace-driven):
  - Front DMAs are issued in need-time order across two HWDGE paths (Sync +
    Scalar) with the critical first chunks small, so the first matmul starts
    ~11us into the kernel (vs ~34us with one monolithic weight DMA) and the
    PE then runs gap-free at the bf16 streaming roofline (~213ns per
    128x128x512 matmul).
  - Capacity is padded to a multiple of 128 (not 512); the remainder tokens
    ride along the last full tile's GEMM1 k-chains into a second PSUM bank.
  - b2 is applied on the host (exact, and it is zero in this module anyway);
    outputs are stored as bf16, halving the output DMA traffic.

Problem shapes (hardcoded per contract): B=4, S=4096, H=1024, F=2048, E=8,
TOPK=2.
"""

import sys

for _p in ("/opt/trn_rl_repo", "/opt/pypackages"):
    if _p not in sys.path:
        sys.path.insert(0, _p)

import ml_dtypes
import numpy as np

import concourse.bass as bass  # noqa: F401  (engine types come via bacc)
import concourse.mybir as mybir
import concourse.tile as tile
from concourse import bacc
from concourse.bass_utils import run_bass_kernel_spmd

H = 1024
F = 2048
E = 8
TOPK = 2
N_CORES = 8
P = 128
NTILE = 512          # full token-tile width (matmul moving dim)
KK1 = H // P         # 8  k-steps in GEMM1
KK2 = F // P         # 16 k-steps in GEMM2
MT1 = F // P         # 16 output m-tiles in GEMM1
NT2 = H // NTILE     # 2  output n-tiles in GEMM2

BF16 = mybir.dt.bfloat16
F32 = mybir.dt.float32

_CACHE = {}
last_result = None   # BassKernelResults of the most recent device run


def _build(C):
    """Build + compile the per-core program for capacity C (multiple of 16)."""
    assert C % 16 == 0
    nfull = C // NTILE            # full 512-token tiles
    rem = C - nfull * NTILE       # remainder width (< 512, any multiple of 16)
    # The remainder rides along with the last full tile: each GEMM1 k-step
    # loads the stationary weight once and streams 512+rem tokens through it
    # as two chained matmuls into separate PSUM banks (the second LDWEIGHTS
    # is pulled into the background weight buffer under the 512-wide stream,
    # so the rider tokens cost only their streaming cycles).
    if nfull == 0:
        widths = [rem]
    elif rem:
        widths = [NTILE] * (nfull - 1) + [NTILE + rem]
    else:
        widths = [NTILE] * nfull
    nmc = -(-C // P)              # token m-chunks (GEMM2 output rows)

    nc = bacc.Bacc("TRN2", target_bir_lowering=False, debug=False,
                   num_devices=N_CORES)

    # DRAM I/O. Layouts are chosen so every DMA is a plain strided copy:
    #   xt[p, kk, c]       = X[c, kk*128+p]   (tokens transposed, H in 8x128)
    #   w1[p, m, kk*128+j] = W1[kk*128+p, m*128+j]  (per-m contiguous chunks)
    #   w2[p, kk, j]       = W2[kk*128+p, j]
    #   b1[p, m]           = b1[m*128+p]
    #   rw[p, mc]          = rweight[mc*128+p]
    #   y[mc, p, j]        = Y[mc*128+p, j]   (bf16)
    xt_d = nc.dram_tensor("xt", [P, KK1, C], BF16, kind="ExternalInput")
    w1_d = nc.dram_tensor("w1", [P, MT1, KK1 * P], BF16, kind="ExternalInput")
    w2_d = nc.dram_tensor("w2", [P, KK2, H], BF16, kind="ExternalInput")
    b1_d = nc.dram_tensor("b1", [P, MT1], F32, kind="ExternalInput")
    rw_d = nc.dram_tensor("rw", [P, nmc], F32, kind="ExternalInput")
    y_d = nc.dram_tensor("y", [nmc, P, H], BF16, kind="ExternalOutput")

    with tile.TileContext(nc) as tc:
        with (
            tc.tile_pool(name="const", bufs=1) as const,
            tc.tile_pool(name="xin", bufs=3) as xin,
            tc.tile_pool(name="gact", bufs=3) as gact,
            tc.tile_pool(name="yout", bufs=4) as yout,
            tc.tile_pool(name="psg", bufs=3, space="PSUM") as psg,
            tc.tile_pool(name="psgr", bufs=2, space="PSUM") as psgr,
            tc.tile_pool(name="psy", bufs=3, space="PSUM") as psy,
        ):
            # First x tile in 4 chunks (2 k-slices each) so GEMM1 m=0 can
            # start as soon as the first 256 KB lands. DMA issue on the Sync
            # engine serializes at ~0.6us per dma_start, so the front only
            # carries the chunks that actually gate compute; the bulk of the
            # weights goes out as a few large transfers.
            # Front DMA schedule. Issue order doubles as packet priority in
            # the SDMA engine FIFOs, so transfers are issued strictly in
            # need-time order and the bulk weights go out last. Sync engine:
            # the xt0 chunks, then the tail of W1, then rw, then W2 (split so
            # its packets queue behind the critical front and the 9th+ sync
            # DMA is gated by queue reuse). Scalar engine (parallel HWDGE
            # path): w1 m0-3 + b1, which gate the first matmuls/eviction.
            # PE clock prewarm: the HAM clock gate needs ~3.4us of sustained
            # matmul activity to lift the PE from 1.2 to 2.4 GHz. Run dummy
            # matmuls on a memset tile while the first input DMAs are in
            # flight so the real matmuls start at full clock.
            dum = const.tile([P, NTILE], BF16)
            nc.gpsimd.memset(dum[:], 0.0)
            pyd = psy.tile([P, NTILE], F32, tag="py")
            for _ in range(14):
                nc.tensor.matmul(pyd[:], dum[:, :P], dum[:],
                                 start=True, stop=True)

            xt0 = xin.tile([P, KK1, NTILE if widths else P], BF16, tag="xt")
            for q in range(4):
                nc.sync.dma_start(xt0[:, 2 * q:2 * q + 2, :],
                                  xt_d[:, 2 * q:2 * q + 2, :widths[0]])

            w1_t = []
            for m in range(4):
                t = const.tile([P, KK1 * P], BF16, tag=f"w1_{m}")
                nc.scalar.dma_start(t[:], w1_d[:, m, :])
                w1_t.append(t)
            b1_sb = const.tile([P, MT1], F32)
            nc.scalar.dma_start(b1_sb[:], b1_d[:])
            # m4/m5 as individual chunks too: they are needed at ~20us and a
            # bulk transfer's latency under front contention can miss that.
            for m in range(4, 6):
                t = const.tile([P, KK1 * P], BF16, tag=f"w1_{m}")
                nc.scalar.dma_start(t[:], w1_d[:, m, :])
                w1_t.append(t)

            # Rest of W1 in two 1.25 MB transfers (needed ~28us+ in).
            w1g = []
            for g in range(2):
                t = const.tile([P, 5, KK1 * P], BF16, tag=f"w1g_{g}")
                nc.sync.dma_start(t[:], w1_d[:, 6 + 5 * g:11 + 5 * g, :])
                w1g.append(t)

            def w1s(m, kk):
                if m < 6:
                    return w1_t[m][:, kk * P:(kk + 1) * P]
                g, mi = divmod(m - 6, 5)
                return w1g[g][:, mi, kk * P:(kk + 1) * P]

            rw_sb = const.tile([P, nmc], F32)
            nc.sync.dma_start(rw_sb[:], rw_d[:])

            # W2 in four 1 MB chunks (needed only when GEMM2 starts ~45us).
            w2_sb = const.tile([P, KK2, H], BF16)
            for g in range(4):
                nc.sync.dma_start(w2_sb[:, 4 * g:4 * g + 4, :],
                                  w2_d[:, 4 * g:4 * g + 4, :])

            cbase = 0
            for ct, w in enumerate(widths):
                if ct == 0:
                    xt_sb = xt0
                else:
                    xt_sb = xin.tile([P, KK1, w], BF16, tag="xt")
                    nc.sync.dma_start(
                        xt_sb[:], xt_d[:, :, cbase:cbase + w])

                # GEMM1: GT[f, c] = sum_h W1[h, f] * XT[h, c], then
                # gelu(.+b1) on eviction. F on partitions, tokens on free.
                wf = min(w, NTILE)       # main chain width
                wr = w - wf              # rider chain width (last tile only)
                gt_sb = gact.tile([P, MT1, w], BF16, tag="gt")
                m_start = 0
                if ct == 0 and wr == 0:
                    # The front is DMA-bandwidth-bound: xt0 k-slices arrive
                    # about every 750ns while a single chain consumes one
                    # every 213ns. Interleave the first 4 m-chains across 4
                    # PSUM banks so each arriving slice is consumed 4x and
                    # the PE never outruns the chunked input DMAs.
                    NI = 4
                    pgs = [psg.tile([P, wf], F32, tag="pg", name=f"pgi{i}")
                           for i in range(NI - 1)]
                    pgs.append(psgr.tile([P, wf], F32, tag="pgr",
                                         name="pgi3"))
                    for kk in range(KK1):
                        for mi in range(NI):
                            nc.tensor.matmul(
                                pgs[mi][:],
                                w1s(mi, kk),
                                xt_sb[:, kk, :wf],
                                start=(kk == 0), stop=(kk == KK1 - 1))
                    for mi in range(NI):
                        nc.scalar.activation(
                            gt_sb[:, mi, :wf], pgs[mi][:],
                            mybir.ActivationFunctionType.Gelu,
                            bias=b1_sb[:, mi:mi + 1])
                    m_start = NI
                for m in range(m_start, MT1):
                    pg = psg.tile([P, wf], F32, tag="pg")
                    if wr:
                        pgr = psgr.tile([P, wr], F32, tag="pgr")
                    for kk in range(KK1):
                        nc.tensor.matmul(
                            pg[:],
                            w1s(m, kk),
                            xt_sb[:, kk, :wf],
                            start=(kk == 0), stop=(kk == KK1 - 1))
                        if wr:
                            nc.tensor.matmul(
                                pgr[:],
                                w1s(m, kk),
                                xt_sb[:, kk, wf:w],
                                start=(kk == 0), stop=(kk == KK1 - 1))
                    nc.scalar.activation(
                        gt_sb[:, m, :wf], pg[:],
                        mybir.ActivationFunctionType.Gelu,
                        bias=b1_sb[:, m:m + 1])
                    if wr:
                        nc.scalar.activation(
                            gt_sb[:, m, wf:w], pgr[:],
                            mybir.ActivationFunctionType.Gelu,
                            bias=b1_sb[:, m:m + 1])

                # GEMM2: Y[c, j] = sum_f GT[f, c] * W2[f, j]; tokens on
                # partitions. Evict: * routing_weight -> bf16, store.
                for mo in range(-(-w // P)):
                    pw = min(P, w - mo * P)   # last chunk may be partial
                    mc = cbase // P + mo
                    for n in range(NT2):
                        py = psy.tile([P, NTILE], F32, tag="py")
                        for kk in range(KK2):
                            nc.tensor.matmul(
                                py[:pw, :],
                                gt_sb[:, kk, mo * P:mo * P + pw],
                                w2_sb[:, kk, n * NTILE:(n + 1) * NTILE],
                                start=(kk == 0), stop=(kk == KK2 - 1))
                        yo = yout.tile([P, NTILE], BF16, tag="yo")
                        nc.vector.tensor_scalar_mul(
                            yo[:pw, :], py[:pw, :], rw_sb[:pw, mc:mc + 1])
                        nc.sync.dma_start(
                            y_d[mc, :pw, n * NTILE:(n + 1) * NTILE],
                            yo[:pw, :])
                cbase += w

    nc.compile()
    return nc


def kernel(hidden_states, expert_weights, top_experts, w1, b1, w2, b2,
           _trace=False):
    global last_result
    x = np.asarray(hidden_states, dtype=np.float32)
    fw = np.asarray(expert_weights, dtype=np.float32).reshape(-1)
    fe = np.asarray(top_experts).reshape(-1).astype(np.int64)
    w1 = np.asarray(w1, dtype=np.float32)
    b1 = np.asarray(b1, dtype=np.float32)
    w2 = np.asarray(w2, dtype=np.float32)
    b2 = np.asarray(b2, dtype=np.float32)

    b, s, h = x.shape
    T = b * s
    xf = x.reshape(T, h)
    npair = T * TOPK

    # Host-side routing: stable sort of pair indices by expert id.
    order = np.argsort(fe, kind="stable")
    counts = np.bincount(fe, minlength=E)
    starts = np.concatenate([[0], np.cumsum(counts)])
    C = max(int(-(-counts.max() // 16)) * 16, 16)
    nmc = -(-C // P)

    key = C
    if key not in _CACHE:
        _CACHE[key] = _build(C)
    nc = _CACHE[key]

    in_maps = []
    for e in range(E):
        idx = order[starts[e]:starts[e + 1]]
        cnt = len(idx)
        xe = np.zeros((C, H), np.float32)
        xe[:cnt] = xf[idx // TOPK]
        xt = np.ascontiguousarray(
            xe.T.reshape(KK1, P, C).transpose(1, 0, 2)).astype(
                ml_dtypes.bfloat16)
        rwe = np.zeros(nmc * P, np.float32)
        rwe[:cnt] = fw[idx]
        in_maps.append({
            "xt": xt,
            "w1": np.ascontiguousarray(
                w1[e].reshape(KK1, P, MT1, P).transpose(1, 2, 0, 3).reshape(
                    P, MT1, KK1 * P)).astype(ml_dtypes.bfloat16),
            "w2": np.ascontiguousarray(
                w2[e].reshape(KK2, P, H).transpose(1, 0, 2)).astype(
                    ml_dtypes.bfloat16),
            "b1": np.ascontiguousarray(b1[e].reshape(MT1, P).T),
            "rw": np.ascontiguousarray(rwe.reshape(nmc, P).T),
        })

    res = run_bass_kernel_spmd(nc, in_maps, list(range(N_CORES)),
                               trace=_trace)
    last_result = res

    routed = np.zeros((npair, H), np.float32)
    for e in range(E):
        idx = order[starts[e]:starts[e + 1]]
        cnt = len(idx)
        ye = np.asarray(res.results[e]["y"]).astype(
            np.float32).reshape(nmc * P, H)
        routed[idx] = ye[:cnt]
        if b2[e].any():
            # b2 is applied on the host (exact fp32): y += rw * b2[expert]
            routed[idx] += fw[idx][:, None] * b2[e][None, :]

    y = routed.reshape(T, TOPK, H).sum(axis=1)
    return y.reshape(b, s, h).astype(np.float32)


# revision 23
# speedup vs baseline: 1.1942x; 1.0037x over previous
"""Grouped-GEMM MoE kernel for Trainium2, expert-parallel across 8 NeuronCores.

Strategy (matches the module's expert-parallel path):
  - Host routes: sort the T*topk routed pairs by expert id; core e gets the
    tokens routed to expert e (padded to a common capacity C).
  - Device (per core): Y = gelu(X @ W1 + b1) @ W2 + b2, scaled per-row by the
    routing weight. Two chained GEMMs on the PE array in bf16 with fp32 PSUM
    accumulation; gelu fused into the PSUM->SBUF eviction on the ACT engine.
  - Host combines: scatter rows back by pair index and sum the topk=2 slots.

Perf notes (trace-driven):
  - Front DMAs are issued in need-time order across two HWDGE paths (Sync +
    Scalar) with the critical first chunks small, so the first matmul starts
    ~11us into the kernel (vs ~34us with one monolithic weight DMA) and the
    PE then runs gap-free at the bf16 streaming roofline (~213ns per
    128x128x512 matmul).
  - Capacity is padded to a multiple of 128 (not 512); the remainder tokens
    ride along the last full tile's GEMM1 k-chains into a second PSUM bank.
  - b2 is applied on the host (exact, and it is zero in this module anyway);
    outputs are stored as bf16, halving the output DMA traffic.

Problem shapes (hardcoded per contract): B=4, S=4096, H=1024, F=2048, E=8,
TOPK=2.
"""

import sys

for _p in ("/opt/trn_rl_repo", "/opt/pypackages"):
    if _p not in sys.path:
        sys.path.insert(0, _p)

import ml_dtypes
import numpy as np

import concourse.bass as bass  # noqa: F401  (engine types come via bacc)
import concourse.mybir as mybir
import concourse.tile as tile
from concourse import bacc
from concourse.bass_utils import run_bass_kernel_spmd

H = 1024
F = 2048
E = 8
TOPK = 2
N_CORES = 8
P = 128
NTILE = 512          # full token-tile width (matmul moving dim)
KK1 = H // P         # 8  k-steps in GEMM1
KK2 = F // P         # 16 k-steps in GEMM2
MT1 = F // P         # 16 output m-tiles in GEMM1
NT2 = H // NTILE     # 2  output n-tiles in GEMM2

BF16 = mybir.dt.bfloat16
F32 = mybir.dt.float32

_CACHE = {}
last_result = None   # BassKernelResults of the most recent device run


def _build(C):
    """Build + compile the per-core program for capacity C (multiple of 16)."""
    assert C % 16 == 0
    nfull = C // NTILE            # full 512-token tiles
    rem = C - nfull * NTILE       # remainder width (< 512, any multiple of 16)
    # The remainder rides along with the last full tile: each GEMM1 k-step
    # loads the stationary weight once and streams 512+rem tokens through it
    # as two chained matmuls into separate PSUM banks (the second LDWEIGHTS
    # is pulled into the background weight buffer under the 512-wide stream,
    # so the rider tokens cost only their streaming cycles).
    if nfull == 0:
        widths = [rem]
    elif rem:
        widths = [NTILE] * (nfull - 1) + [NTILE + rem]
    else:
        widths = [NTILE] * nfull
    nmc = -(-C // P)              # token m-chunks (GEMM2 output rows)

    nc = bacc.Bacc("TRN2", target_bir_lowering=False, debug=False,
                   num_devices=N_CORES)

    # DRAM I/O. Layouts are chosen so every DMA is a plain strided copy:
    #   xt[p, kk, c]       = X[c, kk*128+p]   (tokens transposed, H in 8x128)
    #   w1[p, m, kk*128+j] = W1[kk*128+p, m*128+j]  (per-m contiguous chunks)
    #   w2[p, kk, j]       = W2[kk*128+p, j]
    #   b1[p, m]           = b1[m*128+p]
    #   rw[p, mc]          = rweight[mc*128+p]
    #   y[mc, p, j]        = Y[mc*128+p, j]   (bf16)
    xt_d = nc.dram_tensor("xt", [P, KK1, C], BF16, kind="ExternalInput")
    w1_d = nc.dram_tensor("w1", [P, MT1, KK1 * P], BF16, kind="ExternalInput")
    w2_d = nc.dram_tensor("w2", [P, KK2, H], BF16, kind="ExternalInput")
    b1_d = nc.dram_tensor("b1", [P, MT1], F32, kind="ExternalInput")
    rw_d = nc.dram_tensor("rw", [P, nmc], F32, kind="ExternalInput")
    y_d = nc.dram_tensor("y", [nmc, P, H], BF16, kind="ExternalOutput")

    with tile.TileContext(nc) as tc:
        with (
            tc.tile_pool(name="const", bufs=1) as const,
            tc.tile_pool(name="xin", bufs=3) as xin,
            tc.tile_pool(name="gact", bufs=3) as gact,
            tc.tile_pool(name="yout", bufs=4) as yout,
            tc.tile_pool(name="psg", bufs=3, space="PSUM") as psg,
            tc.tile_pool(name="psgr", bufs=2, space="PSUM") as psgr,
            tc.tile_pool(name="psy", bufs=3, space="PSUM") as psy,
        ):
            # First x tile in 4 chunks (2 k-slices each) so GEMM1 m=0 can
            # start as soon as the first 256 KB lands. DMA issue on the Sync
            # engine serializes at ~0.6us per dma_start, so the front only
            # carries the chunks that actually gate compute; the bulk of the
            # weights goes out as a few large transfers.
            # Front DMA schedule. Issue order doubles as packet priority in
            # the SDMA engine FIFOs, so transfers are issued strictly in
            # need-time order and the bulk weights go out last. Sync engine:
            # the xt0 chunks, then the tail of W1, then rw, then W2 (split so
            # its packets queue behind the critical front and the 9th+ sync
            # DMA is gated by queue reuse). Scalar engine (parallel HWDGE
            # path): w1 m0-3 + b1, which gate the first matmuls/eviction.
            # PE clock prewarm: the HAM clock gate needs ~3.4us of sustained
            # matmul activity to lift the PE from 1.2 to 2.4 GHz. Run dummy
            # matmuls on a memset tile while the first input DMAs are in
            # flight so the real matmuls start at full clock.
            dum = const.tile([P, NTILE], BF16)
            nc.gpsimd.memset(dum[:], 0.0)
            pyd = psy.tile([P, NTILE], F32, tag="py")
            for _ in range(14):
                nc.tensor.matmul(pyd[:], dum[:, :P], dum[:],
                                 start=True, stop=True)

            xt0 = xin.tile([P, KK1, NTILE if widths else P], BF16, tag="xt")
            for q in range(4):
                nc.sync.dma_start(xt0[:, 2 * q:2 * q + 2, :],
                                  xt_d[:, 2 * q:2 * q + 2, :widths[0]])

            w1_t = []
            for m in range(4):
                t = const.tile([P, KK1 * P], BF16, tag=f"w1_{m}")
                nc.scalar.dma_start(t[:], w1_d[:, m, :])
                w1_t.append(t)
            b1_sb = const.tile([P, MT1], F32)
            nc.scalar.dma_start(b1_sb[:], b1_d[:])
            # m4/m5 as individual chunks too: they are needed at ~20us and a
            # bulk transfer's latency under front contention can miss that.
            for m in range(4, 6):
                t = const.tile([P, KK1 * P], BF16, tag=f"w1_{m}")
                nc.scalar.dma_start(t[:], w1_d[:, m, :])
                w1_t.append(t)

            # Rest of W1 also as per-m 256 KB chunks (on Sync): small
            # transfers complete individually and early, where a 1.25 MB
            # group's tail latency under front contention can miss the
            # consuming matmul's need time.
            for m in range(6, MT1):
                t = const.tile([P, KK1 * P], BF16, tag=f"w1_{m}")
                nc.sync.dma_start(t[:], w1_d[:, m, :])
                w1_t.append(t)

            def w1s(m, kk):
                return w1_t[m][:, kk * P:(kk + 1) * P]

            rw_sb = const.tile([P, nmc], F32)
            nc.sync.dma_start(rw_sb[:], rw_d[:])

            # W2 in four 1 MB chunks (needed only when GEMM2 starts ~45us).
            w2_sb = const.tile([P, KK2, H], BF16)
            for g in range(4):
                nc.sync.dma_start(w2_sb[:, 4 * g:4 * g + 4, :],
                                  w2_d[:, 4 * g:4 * g + 4, :])

            cbase = 0
            for ct, w in enumerate(widths):
                if ct == 0:
                    xt_sb = xt0
                else:
                    xt_sb = xin.tile([P, KK1, w], BF16, tag="xt")
                    nc.sync.dma_start(
                        xt_sb[:], xt_d[:, :, cbase:cbase + w])

                # GEMM1: GT[f, c] = sum_h W1[h, f] * XT[h, c], then
                # gelu(.+b1) on eviction. F on partitions, tokens on free.
                wf = min(w, NTILE)       # main chain width
                wr = w - wf              # rider chain width (last tile only)
                gt_sb = gact.tile([P, MT1, w], BF16, tag="gt")
                m_start = 0
                if ct == 0 and wr == 0:
                    # The front is DMA-bandwidth-bound: xt0 k-slices arrive
                    # about every 750ns while a single chain consumes one
                    # every 213ns. Interleave the first 4 m-chains across 4
                    # PSUM banks so each arriving slice is consumed 4x and
                    # the PE never outruns the chunked input DMAs.
                    NI = 4
                    pgs = [psg.tile([P, wf], F32, tag="pg", name=f"pgi{i}")
                           for i in range(NI - 1)]
                    pgs.append(psgr.tile([P, wf], F32, tag="pgr",
                                         name="pgi3"))
                    for kk in range(KK1):
                        for mi in range(NI):
                            nc.tensor.matmul(
                                pgs[mi][:],
                                w1s(mi, kk),
                                xt_sb[:, kk, :wf],
                                start=(kk == 0), stop=(kk == KK1 - 1))
                    for mi in range(NI):
                        nc.scalar.activation(
                            gt_sb[:, mi, :wf], pgs[mi][:],
                            mybir.ActivationFunctionType.Gelu,
                            bias=b1_sb[:, mi:mi + 1])
                    m_start = NI
                for m in range(m_start, MT1):
                    pg = psg.tile([P, wf], F32, tag="pg")
                    if wr:
                        pgr = psgr.tile([P, wr], F32, tag="pgr")
                    for kk in range(KK1):
                        nc.tensor.matmul(
                            pg[:],
                            w1s(m, kk),
                            xt_sb[:, kk, :wf],
                            start=(kk == 0), stop=(kk == KK1 - 1))
                        if wr:
                            nc.tensor.matmul(
                                pgr[:],
                                w1s(m, kk),
                                xt_sb[:, kk, wf:w],
                                start=(kk == 0), stop=(kk == KK1 - 1))
                    nc.scalar.activation(
                        gt_sb[:, m, :wf], pg[:],
                        mybir.ActivationFunctionType.Gelu,
                        bias=b1_sb[:, m:m + 1])
                    if wr:
                        nc.scalar.activation(
                            gt_sb[:, m, wf:w], pgr[:],
                            mybir.ActivationFunctionType.Gelu,
                            bias=b1_sb[:, m:m + 1])

                # GEMM2: Y[c, j] = sum_f GT[f, c] * W2[f, j]; tokens on
                # partitions. Evict: * routing_weight -> bf16, store.
                for mo in range(-(-w // P)):
                    pw = min(P, w - mo * P)   # last chunk may be partial
                    mc = cbase // P + mo
                    for n in range(NT2):
                        py = psy.tile([P, NTILE], F32, tag="py")
                        for kk in range(KK2):
                            nc.tensor.matmul(
                                py[:pw, :],
                                gt_sb[:, kk, mo * P:mo * P + pw],
                                w2_sb[:, kk, n * NTILE:(n + 1) * NTILE],
                                start=(kk == 0), stop=(kk == KK2 - 1))
                        yo = yout.tile([P, NTILE], BF16, tag="yo")
                        nc.vector.tensor_scalar_mul(
                            yo[:pw, :], py[:pw, :], rw_sb[:pw, mc:mc + 1])
                        nc.sync.dma_start(
                            y_d[mc, :pw, n * NTILE:(n + 1) * NTILE],
                            yo[:pw, :])
                cbase += w

    nc.compile()
    return nc


def kernel(hidden_states, expert_weights, top_experts, w1, b1, w2, b2,
           _trace=False):
    global last_result
    x = np.asarray(hidden_states, dtype=np.float32)
    fw = np.asarray(expert_weights, dtype=np.float32).reshape(-1)
    fe = np.asarray(top_experts).reshape(-1).astype(np.int64)
    w1 = np.asarray(w1, dtype=np.float32)
    b1 = np.asarray(b1, dtype=np.float32)
    w2 = np.asarray(w2, dtype=np.float32)
    b2 = np.asarray(b2, dtype=np.float32)

    b, s, h = x.shape
    T = b * s
    xf = x.reshape(T, h)
    npair = T * TOPK

    # Host-side routing: stable sort of pair indices by expert id.
    order = np.argsort(fe, kind="stable")
    counts = np.bincount(fe, minlength=E)
    starts = np.concatenate([[0], np.cumsum(counts)])
    C = max(int(-(-counts.max() // 16)) * 16, 16)
    nmc = -(-C // P)

    key = C
    if key not in _CACHE:
        _CACHE[key] = _build(C)
    nc = _CACHE[key]

    in_maps = []
    for e in range(E):
        idx = order[starts[e]:starts[e + 1]]
        cnt = len(idx)
        xe = np.zeros((C, H), np.float32)
        xe[:cnt] = xf[idx // TOPK]
        xt = np.ascontiguousarray(
            xe.T.reshape(KK1, P, C).transpose(1, 0, 2)).astype(
                ml_dtypes.bfloat16)
        rwe = np.zeros(nmc * P, np.float32)
        rwe[:cnt] = fw[idx]
        in_maps.append({
            "xt": xt,
            "w1": np.ascontiguousarray(
                w1[e].reshape(KK1, P, MT1, P).transpose(1, 2, 0, 3).reshape(
                    P, MT1, KK1 * P)).astype(ml_dtypes.bfloat16),
            "w2": np.ascontiguousarray(
                w2[e].reshape(KK2, P, H).transpose(1, 0, 2)).astype(
                    ml_dtypes.bfloat16),
            "b1": np.ascontiguousarray(b1[e].reshape(MT1, P).T),
            "rw": np.ascontiguousarray(rwe.reshape(nmc, P).T),
        })

    res = run_bass_kernel_spmd(nc, in_maps, list(range(N_CORES)),
                               trace=_trace)
    last_result = res

    routed = np.zeros((npair, H), np.float32)
    for e in range(E):
        idx = order[starts[e]:starts[e + 1]]
        cnt = len(idx)
        ye = np.asarray(res.results[e]["y"]).astype(
            np.float32).reshape(nmc * P, H)
        routed[idx] = ye[:cnt]
        if b2[e].any():
            # b2 is applied on the host (exact fp32): y += rw * b2[expert]
            routed[idx] += fw[idx][:, None] * b2[e][None, :]

    y = routed.reshape(T, TOPK, H).sum(axis=1)
    return y.reshape(b, s, h).astype(np.float32)
